# revision 41
# baseline (speedup 1.0000x reference)
"""AttentionDecoderRNN single-step decoder on 8 Trainium2 NeuronCores.

Math (faithful to the reference, including log-softmax attention weights):
  emb = embedding[word] ; x = [emb, last_context]
  GRU step (gate order r,z,n) -> h'
  u = Wa^T h' ; c0 = ba . h' ; E[s] = enc[s,:] . u + c0
  attw = log_softmax(E)
  ctx = attw @ enc = P - lse(E)*Q,  P[k] = sum_s E[s] enc[s,k], Q[k] = sum_s enc[s,k]
  out = log_softmax([h', ctx] @ Wo^T + bo)

Sharding (8 cores):
  - GRU gate dim sharded (128 gates/core) -> AllGather h'
  - encoder seq sharded 512/core; E, P, Q partials -> one AllGather
  - Wo sharded by vocab rows (6656 padded per core), bf16, streamed in two
    halves (h-part early, ctx-part late) -> AllGather logits -> global
    log-softmax on every core.
Small matvecs use stationary-weights orientation (out [128,1]); the big Wo
stream uses moving-weights orientation (out [1,512], bf16).
"""

import numpy as np

H = 1024
V = 50257
S = 4096
C = 8
VLOC = 6288          # 12*512 + 144; VPAD multiple of 128
VPAD = C * VLOC      # 50304
NCH = 13
NCOL = 512
NCOL_LAST = VLOC - 12 * NCOL   # 144
CHOFF = [ch * NCOL for ch in range(12)] + [12 * NCOL]
CHW = [NCOL] * 12 + [NCOL_LAST]
SLOC = S // C        # 512

_PROG = None


def _build_program():
    import concourse.bacc as bacc
    import concourse.bass as bass
    import concourse.tile as tile
    import concourse.mybir as mybir
    from concourse import bass_isa

    f32 = mybir.dt.float32
    bf16 = mybir.dt.float16
    i32 = mybir.dt.int32
    AF = mybir.ActivationFunctionType
    OP = mybir.AluOpType
    AX = mybir.AxisListType

    nc = bacc.Bacc(None, target_bir_lowering=False)

    # ---- inputs ----
    word = nc.dram_tensor("word", [1], i32, kind="ExternalInput")
    emb = nc.dram_tensor("emb", [V, H], f32, kind="ExternalInput")
    lchc = nc.dram_tensor("lchc", [128, 17], f32, kind="ExternalInput")
    rowpack = nc.dram_tensor("rowpack", [1, 1024], f32, kind="ExternalInput")
    wih = nc.dram_tensor("wih", [128, 16, 3, 128], f32, kind="ExternalInput")
    whh = nc.dram_tensor("whh", [128, 8, 3, 128], f32, kind="ExternalInput")
    wa = nc.dram_tensor("wa", [128, 8, 8, 128], f32, kind="ExternalInput")
    ba_col = nc.dram_tensor("ba_col", [128, 8], f32, kind="ExternalInput")
    encT = nc.dram_tensor("encT", [128, 8, 4, 128], f32, kind="ExternalInput")
    encN = nc.dram_tensor("encN", [128, 4, 8, 128], f32, kind="ExternalInput")
    wod_h = nc.dram_tensor("wod_h", [12, 128, 8, NCOL], bf16, kind="ExternalInput")
    wod_c = nc.dram_tensor("wod_c", [12, 128, 8, NCOL], bf16, kind="ExternalInput")
    wod_hl = nc.dram_tensor("wod_hl", [128, 8, NCOL_LAST], bf16, kind="ExternalInput")
    wod_cl = nc.dram_tensor("wod_cl", [128, 8, NCOL_LAST], bf16, kind="ExternalInput")
    bo_r = nc.dram_tensor("bo_r", [VLOC], f32, kind="ExternalInput")

    # ---- outputs ----
    out_logp = nc.dram_tensor("out_logp", [VPAD], f32, kind="ExternalOutput")
    out_ctx = nc.dram_tensor("out_ctx", [H], f32, kind="ExternalOutput")
    out_hidden = nc.dram_tensor("out_hidden", [H], f32, kind="ExternalOutput")
    out_attw = nc.dram_tensor("out_attw", [S], f32, kind="ExternalOutput")

    # ---- internal DRAM ----
    hb_in = nc.dram_tensor("hb_in", [128], f32)
    hb_out = nc.dram_tensor("hb_out", [C * 128], f32, addr_space="Shared")
    eb_in = nc.dram_tensor("eb_in", [2560], f32)
    eb_out = nc.dram_tensor("eb_out", [C, 2560], f32, addr_space="Shared")
    lb_in = nc.dram_tensor("lb_in", [VLOC], f32)
    lb_out = nc.dram_tensor("lb_out", [VPAD], f32, addr_space="Shared")

    RG = [list(range(C))]

    def r1(t):
        return t.rearrange("(a j) -> a j", a=1)

    with tile.TileContext(nc) as tc:
        with (
            tc.tile_pool(name="sb", bufs=1) as sb,
            tc.tile_pool(name="sb2", bufs=2) as sb2,
            tc.tile_pool(name="woh", bufs=3) as wohp,
            tc.tile_pool(name="woc", bufs=3) as wocp,
            tc.tile_pool(name="ps2", bufs=2, space="PSUM") as ps2,
            tc.tile_pool(name="ps4", bufs=4, space="PSUM") as ps4,
        ):
            # ---------- loads: tiny GRU-critical first, same HWDGE ring ----
            w1 = sb.tile([1, 1], i32)
            nc.scalar.dma_start(w1[:], r1(word))
            x_col = sb.tile([128, 25], f32)   # emb | lc | hc | own-h-slice
            nc.scalar.dma_start(x_col[:, 8:25], lchc[:])
            rp_t = sb.tile([1, 1024], f32)
            nc.scalar.dma_start(rp_t[:], rowpack[:])

            w2 = sb.tile([2, 1], i32)
            nc.gpsimd.partition_broadcast(w2[:], w1[:], channels=2)
            erow = sb.tile([2, H], f32, tag="scr2")
            idma = nc.gpsimd.indirect_dma_start(
                out=erow[:], out_offset=None, in_=emb[:],
                in_offset=bass.IndirectOffsetOnAxis(ap=w2[:, 0:1], axis=0),
            )

            ones11 = sb.tile([1, 1], f32)
            nc.vector.memset(ones11[:], 1.0)
            # emb row -> x_col columns via 8 PE transposes (no DMA hops)
            for t in range(8):
                pt = ps2.tile([128, 1], f32, tag="mv")
                nc.tensor.transpose(pt[:], erow[0:1, t * 128:(t + 1) * 128],
                                    ones11[:])
                nc.vector.tensor_copy(x_col[:, t:t + 1], pt[:])

            wih_t = sb.tile([128, 16, 3, 128], f32)
            wih_i = nc.scalar.dma_start(wih_t[:], wih[:])
            bass._add_dep_helper(wih_i.ins, idma.ins, sync=True,
                                 reason="let the tiny emb gather go first")
            whh_t = sb.tile([128, 8, 3, 128], f32)
            encN_t = sb.tile([128, 4, 8, 128], f32)
            whh_i = nc.scalar.dma_start(whh_t[:], whh[:])
            bass._add_dep_helper(whh_i.ins, wih_i.ins, sync=False,
                                 reason="stream order")
            encNa_i = nc.sync.dma_start(encN_t[:, 0:2], encN[:, 0:2])
            bass._add_dep_helper(encNa_i.ins, whh_i.ins, sync=True,
                                 reason="fill front gap")
            wa_t = sb.tile([128, 8, 8, 128], f32)
            encT_t = sb.tile([128, 8, 4, 128], f32)
            use_post_agh_loads = True
            bac_t = sb.tile([128, 8], f32)
            nc.scalar.dma_start(bac_t[:], ba_col[:])

            ones_row = sb.tile([1, 128], f32)
            nc.vector.memset(ones_row[:], 1.0)
            ones_col = sb.tile([128, 1], f32)
            nc.vector.memset(ones_col[:], 1.0)

            # ---------- GRU (gates on partitions, out [128,1]) ----------
            # r_pre/z_pre = W_ih_g x + b_ih_g + W_hh_g h + b_hh_g
            rz_pre = sb.tile([128, 2], f32)
            for g in (0, 1):
                pg = ps2.tile([128, 1], f32, tag="mv")
                for kt in range(16):
                    nc.tensor.matmul(pg[:], wih_t[:, kt, g, :],
                                     x_col[:, kt:kt + 1],
                                     start=(kt == 0), stop=False)
                nc.tensor.matmul(pg[:], rp_t[:, g * 128:(g + 1) * 128],
                                 ones11[:], start=False, stop=False)
                for kt in range(8):
                    nc.tensor.matmul(pg[:], whh_t[:, kt, g, :],
                                     x_col[:, 16 + kt:17 + kt],
                                     start=False, stop=False)
                nc.tensor.matmul(pg[:],
                                 rp_t[:, 384 + g * 128:384 + (g + 1) * 128],
                                 ones11[:], start=False, stop=True)
                nc.vector.tensor_copy(rz_pre[:, g:g + 1], pg[:])
            rz = sb.tile([128, 2], f32)
            nc.scalar.activation(rz[:], rz_pre[:], AF.Sigmoid)

            # n gate: n1 = gi_n + b_ih_n ; n2 = gh_n + b_hh_n
            p_n1 = ps2.tile([128, 1], f32, tag="mv")
            for kt in range(16):
                nc.tensor.matmul(p_n1[:], wih_t[:, kt, 2, :],
                                 x_col[:, kt:kt + 1], start=(kt == 0),
                                 stop=False)
            nc.tensor.matmul(p_n1[:], rp_t[:, 256:384], ones11[:],
                             start=False, stop=True)
            n1 = sb.tile([128, 1], f32)
            nc.vector.tensor_copy(n1[:], p_n1[:])
            p_n2 = ps2.tile([128, 1], f32, tag="mv")
            for kt in range(8):
                nc.tensor.matmul(p_n2[:], whh_t[:, kt, 2, :],
                                 x_col[:, 16 + kt:17 + kt], start=(kt == 0),
                                 stop=False)
            nc.tensor.matmul(p_n2[:], rp_t[:, 640:768], ones11[:],
                             start=False, stop=True)

            nt1 = sb.tile([128, 1], f32)
            nc.vector.tensor_tensor(out=nt1[:], in0=rz[:, 0:1], in1=p_n2[:],
                                    op=OP.mult)
            nt2 = sb.tile([128, 1], f32)
            nc.vector.tensor_add(nt2[:], n1[:], nt1[:])
            n_t = sb.tile([128, 1], f32)
            nc.scalar.activation(n_t[:], nt2[:], AF.Tanh)
            # h' = n + z*(h_own - n)
            d_t = sb.tile([128, 1], f32)
            nc.vector.tensor_tensor(out=d_t[:], in0=x_col[:, 24:25],
                                    in1=n_t[:], op=OP.subtract)
            zd = sb.tile([128, 1], f32)
            nc.vector.tensor_tensor(out=zd[:], in0=rz[:, 1:2], in1=d_t[:],
                                    op=OP.mult)
            hn = sb.tile([128, 1], f32)
            nc.vector.tensor_add(hn[:], n_t[:], zd[:])

            hb_i = nc.scalar.dma_start(hb_in.rearrange("(p a) -> p a", a=1),
                                       hn[:])
            nc.gpsimd.collective_compute(
                "AllGather", OP.bypass, replica_groups=RG,
                ins=[hb_in[:]], outs=[hb_out[:]],
            )
            wa_i = nc.sync.dma_start(wa_t[:], wa[:])
            bass._add_dep_helper(wa_i.ins, hb_i.ins, sync=True,
                                 reason="h bounce beats big loads")
            bass._add_dep_helper(wa_i.ins, encNa_i.ins, sync=False,
                                 reason="stream order")
            encN_i = nc.sync.dma_start(encN_t[:, 2:4], encN[:, 2:4])
            bass._add_dep_helper(encN_i.ins, wa_i.ins, sync=False,
                                 reason="stream order")

            # ---------- q[k] = sum_s enc[s,k] during the AllGather ---------
            pq_stage = sb.tile([128, 16], f32)   # col = j*8 + kt
            for kt in range(8):
                pqq = ps2.tile([128, 1], f32, tag="mv")
                for st in range(4):
                    nc.tensor.matmul(pqq[:], encN_t[:, st, kt, :],
                                     ones_col[:], start=(st == 0),
                                     stop=(st == 3))
                nc.vector.tensor_copy(pq_stage[:, 8 + kt:9 + kt], pqq[:])

            h_col = sb.tile([128, 8], f32)
            hcol_i = nc.scalar.dma_start(h_col[:],
                                         hb_out.rearrange("(c p) -> p c", p=128))
            encTa_i = nc.sync.dma_start(encT_t[:, 0:4], encT[:, 0:4])
            bass._add_dep_helper(encTa_i.ins, hcol_i.ins, sync=True,
                                 reason="h reload beats encT")
            bass._add_dep_helper(encTa_i.ins, encN_i.ins, sync=False,
                                 reason="stream order")
            encTb_i = nc.sync.dma_start(encT_t[:, 4:8], encT[:, 4:8])
            bass._add_dep_helper(encTb_i.ins, encTa_i.ins, sync=False,
                                 reason="stream order")
            x2h = sb.tile([128, 8], bf16)
            nc.vector.tensor_copy(x2h[:], h_col[:])
            h_lin = sb.tile([2, H], f32, tag="scr2")
            nc.scalar.dma_start(h_lin[0:1, :], r1(hb_out))
            nc.scalar.dma_start(r1(out_hidden), h_lin[0:1, :])

            # ---------- u = Wa^T h ; c0 = ba . h ----------
            u_col = sb.tile([128, 8], f32)
            for kt in range(8):
                pu = ps2.tile([128, 1], f32, tag="mv")
                for jt in range(8):
                    nc.tensor.matmul(pu[:], wa_t[:, jt, kt, :],
                                     h_col[:, jt:jt + 1],
                                     start=(jt == 0), stop=(jt == 7))
                nc.vector.tensor_copy(u_col[:, kt:kt + 1], pu[:])
            pc0 = ps2.tile([128, 1], f32, tag="mv")
            for jt in range(8):
                nc.tensor.matmul(pc0[0:1, :], bac_t[:, jt:jt + 1],
                                 h_col[:, jt:jt + 1],
                                 start=(jt == 0), stop=(jt == 7))
            c0 = sb.tile([1, 1], f32)
            nc.vector.tensor_copy(c0[:], pc0[0:1, :])

            # ---------- E_loc[s] = enc_c[s,:] . u + c0 ----------
            E_col = sb.tile([128, 4], f32)
            for st in range(4):
                pe = ps2.tile([128, 1], f32, tag="mv")
                for kt in range(8):
                    nc.tensor.matmul(pe[:], encT_t[:, kt, st, :],
                                     u_col[:, kt:kt + 1],
                                     start=(kt == 0), stop=False)
                nc.tensor.matmul(pe[:], ones_row[:], c0[:],
                                 start=False, stop=True)
                nc.vector.tensor_copy(E_col[:, st:st + 1], pe[:])

            # ---------- p[k] = sum_s E[s] enc[s,k] ----------
            for kt in range(8):
                pp = ps2.tile([128, 1], f32, tag="mv")
                for st in range(4):
                    nc.tensor.matmul(pp[:], encN_t[:, st, kt, :],
                                     E_col[:, st:st + 1],
                                     start=(st == 0), stop=(st == 3))
                nc.vector.tensor_copy(pq_stage[:, kt:kt + 1], pp[:])

            # bounce [E | P | Q] (PQ k-linear col-major) and AllGather
            eb1_i = nc.scalar.dma_start(
                eb_in[0:512].rearrange("(p s) -> p s", s=4), E_col[:])
            eb2_i = nc.scalar.dma_start(
                eb_in[512:2560].rearrange("(col p) -> p col", p=128),
                pq_stage[:])
            nc.gpsimd.collective_compute(
                "AllGather", OP.bypass, replica_groups=RG,
                ins=[eb_in[:]], outs=[eb_out[:]],
            )

            # ---------- attention log-softmax stats ----------
            E_all = sb.tile([8, 512], f32)
            nc.scalar.dma_start(E_all[:], eb_out[:, 0:512])
            m8 = sb.tile([8, 1], f32)
            nc.vector.reduce_max(m8[:], E_all[:], axis=AX.X)
            M8 = sb.tile([8, 1], f32)
            nc.gpsimd.partition_all_reduce(M8[:], m8[:], channels=8,
                                           reduce_op=bass_isa.ReduceOp.max)
            negM = sb.tile([8, 1], f32)
            nc.vector.tensor_scalar_mul(negM[:], M8[:], -1.0)
            attw = sb.tile([8, 512], f32)
            nc.scalar.activation(attw[:], E_all[:], AF.Exp, bias=negM[:, 0:1])
            s8 = sb.tile([8, 1], f32)
            nc.vector.reduce_sum(s8[:], attw[:], axis=AX.X)
            S8 = sb.tile([8, 1], f32)
            nc.gpsimd.partition_all_reduce(S8[:], s8[:], channels=8,
                                           reduce_op=bass_isa.ReduceOp.add)
            lnS = sb.tile([8, 1], f32)
            nc.scalar.activation(lnS[:], S8[:], AF.Ln)
            lseE = sb.tile([8, 1], f32)
            nc.vector.tensor_add(lseE[:], M8[:], lnS[:])
            nc.vector.tensor_scalar(out=attw[:], in0=E_all[:],
                                    scalar1=lseE[:, 0:1], scalar2=None,
                                    op0=OP.subtract)
            nc.scalar.dma_start(out_attw.rearrange("(c e) -> c e", e=512),
                                attw[:])
            lse128 = sb.tile([128, 1], f32)
            nc.gpsimd.partition_broadcast(lse128[:], lseE[0:1, :], channels=128)

            # ---------- logits pass 1: h @ Wo_h^T + bo -> Lh_sb ----------
            prev_dma = [encTb_i]
            Lh_sb = sb.tile([1, VLOC], bf16)
            for ch in range(NCH):
                off, w = CHOFF[ch], CHW[ch]
                wt = wohp.tile([128, 8, w], bf16, tag="woh")
                wdma = nc.sync.dma_start(
                    wt[:], wod_h[ch] if ch < 12 else wod_hl[:])
                if ch == 0:
                    bass._add_dep_helper(wdma.ins, eb2_i.ins, sync=True,
                                         reason="E/PQ bounce beats Wo stream")
                    bass._add_dep_helper(wdma.ins, eb1_i.ins, sync=True,
                                         reason="E bounce beats Wo stream")
                else:
                    bass._add_dep_helper(wdma.ins, prev_dma[0].ins, sync=False,
                                         reason="stream order")
                prev_dma[0] = wdma
                bt = sb2.tile([1, w], f32, tag="bo")
                nc.scalar.dma_start(bt[:], r1(bo_r[off:off + w]))
                pL = ps4.tile([1, w], f32, tag="Lp")
                for kt in range(8):
                    nc.tensor.matmul(pL[:], x2h[:, kt:kt + 1], wt[:, kt, :],
                                     start=(kt == 0), stop=(kt == 7))
                nc.vector.tensor_add(Lh_sb[:, off:off + w], pL[:], bt[:])

            # ---------- ctx = P_sum - lse * Q_sum (column layout) ---------
            pq_all = sb.tile([8, 2048], f32)
            nc.scalar.dma_start(pq_all[:], eb_out[:, 512:2560])
            pq_col = sb.tile([128, 16], f32)   # P cols 0-7, Q cols 8-15
            for kc in range(16):
                ps_ = ps2.tile([128, 1], f32, tag="mv")
                nc.tensor.matmul(ps_[:], pq_all[:, kc * 128:(kc + 1) * 128],
                                 ones_col[0:8, :], start=True, stop=True)
                nc.vector.tensor_copy(pq_col[:, kc:kc + 1], ps_[:])
            ctx_col = sb.tile([128, 8], f32)
            nc.vector.tensor_scalar(out=ctx_col[:], in0=pq_col[:, 8:16],
                                    scalar1=lse128[:, 0:1],
                                    scalar2=None, op0=OP.mult)
            nc.vector.tensor_tensor(out=ctx_col[:], in0=pq_col[:, 0:8],
                                    in1=ctx_col[:], op=OP.subtract)
            nc.scalar.dma_start(
                out_ctx.rearrange("(t p) -> p t", p=128), ctx_col[:])
            x2c = sb.tile([128, 8], bf16)
            nc.vector.tensor_copy(x2c[:], ctx_col[:])

            # ---------- logits pass 2: + ctx @ Wo_c^T -> lb_in ----------
            for ch in range(NCH):
                off, w = CHOFF[ch], CHW[ch]
                wt = wocp.tile([128, 8, w], bf16, tag="woc")
                wdma = nc.sync.dma_start(
                    wt[:], wod_c[ch] if ch < 12 else wod_cl[:])
                bass._add_dep_helper(wdma.ins, prev_dma[0].ins, sync=False,
                                     reason="stream order")
                prev_dma[0] = wdma
                pL = ps4.tile([1, w], f32, tag="Lp")
                for kt in range(8):
                    nc.tensor.matmul(pL[:], x2c[:, kt:kt + 1], wt[:, kt, :],
                                     start=(kt == 0), stop=(kt == 7))
                lrow = sb2.tile([1, w], f32, tag="lrow")
                nc.vector.tensor_add(lrow[:], pL[:], Lh_sb[:, off:off + w])
                nc.scalar.dma_start(r1(lb_in[off:off + w]), lrow[:])

            nc.gpsimd.collective_compute(
                "AllGather", OP.bypass, replica_groups=RG,
                ins=[lb_in[:]], outs=[lb_out[:]],
            )

            # ---------- global log-softmax ----------
            F = VPAD // 128
            La = sb.tile([128, F], f32)
            nc.scalar.dma_start(La[:], lb_out.rearrange("(p f) -> p f", p=128))
            mL = sb.tile([128, 1], f32)
            nc.vector.reduce_max(mL[:], La[:], axis=AX.X)
            ML = sb.tile([128, 1], f32)
            nc.gpsimd.partition_all_reduce(ML[:], mL[:], channels=128,
                                           reduce_op=bass_isa.ReduceOp.max)
            negML = sb.tile([128, 1], f32)
            nc.vector.tensor_scalar_mul(negML[:], ML[:], -1.0)
            oL = sb.tile([128, F], f32)
            nc.scalar.activation(oL[:], La[:], AF.Exp, bias=negML[:, 0:1])
            sL = sb.tile([128, 1], f32)
            nc.vector.reduce_sum(sL[:], oL[:], axis=AX.X)
            SL = sb.tile([128, 1], f32)
            nc.gpsimd.partition_all_reduce(SL[:], sL[:], channels=128,
                                           reduce_op=bass_isa.ReduceOp.add)
            lnSL = sb.tile([128, 1], f32)
            nc.scalar.activation(lnSL[:], SL[:], AF.Ln)
            lseL = sb.tile([128, 1], f32)
            nc.vector.tensor_add(lseL[:], ML[:], lnSL[:])
            nc.vector.tensor_scalar(out=oL[:], in0=La[:], scalar1=lseL[:, 0:1],
                                    scalar2=None, op0=OP.subtract)
            nc.scalar.dma_start(out_logp.rearrange("(p f) -> p f", p=128),
                                oL[:])

    nc.compile()
    return nc


def _prep_inputs(word_input, last_context, last_hidden, encoder_outputs,
                 embedding, W_ih, W_hh, b_ih, b_hh, Wa, ba, Wo, bo):
    lc = np.ascontiguousarray(last_context, dtype=np.float32).reshape(H)
    hh = np.ascontiguousarray(last_hidden, dtype=np.float32).reshape(H)
    enc = np.ascontiguousarray(encoder_outputs, dtype=np.float32).reshape(S, H)
    W_ih = np.asarray(W_ih, dtype=np.float32)
    W_hh = np.asarray(W_hh, dtype=np.float32)
    Wa_ = np.asarray(Wa, dtype=np.float32)
    Wo_ = np.asarray(Wo, dtype=np.float32)
    emb = np.ascontiguousarray(embedding, dtype=np.float32)
    b_ih = np.asarray(b_ih, dtype=np.float32)
    b_hh = np.asarray(b_hh, dtype=np.float32)
    ba_ = np.asarray(ba, dtype=np.float32)
    bo_ = np.asarray(bo, dtype=np.float32)

    word = np.asarray(word_input).reshape(1).astype(np.int32)
    lc_col = lc.reshape(8, 128).T
    hc_col = hh.reshape(8, 128).T
    wa_t = np.ascontiguousarray(Wa_.reshape(8, 128, 8, 128).transpose(1, 0, 2, 3))
    ba_col = np.ascontiguousarray(ba_.reshape(8, 128).T)

    in_maps = []
    for c in range(C):
        rows = np.r_[c * 128:(c + 1) * 128,
                     H + c * 128:H + (c + 1) * 128,
                     2 * H + c * 128:2 * H + (c + 1) * 128]
        # wih lhsT layout: [p(k), kt, gate, m] = W_ih[rows[g*128+m], kt*128+p]
        rih = W_ih[rows, :]                 # [384 (g-major), 2048]
        wih_t = np.ascontiguousarray(
            rih.reshape(3, 128, 2048).transpose(2, 0, 1)
               .reshape(16, 128, 3, 128).transpose(1, 0, 2, 3))
        rhhh = W_hh[rows, :]                # [384, 1024]
        whh_t = np.ascontiguousarray(
            rhhh.reshape(3, 128, 1024).transpose(2, 0, 1)
                .reshape(8, 128, 3, 128).transpose(1, 0, 2, 3))
        rowpack = np.zeros((1, 1024), dtype=np.float32)
        rowpack[0, 0:384] = b_ih[rows]
        rowpack[0, 384:768] = b_hh[rows]
        lchc_c = np.zeros((128, 17), dtype=np.float32)
        lchc_c[:, 0:8] = lc_col
        lchc_c[:, 8:16] = hc_col
        lchc_c[:, 16] = hh[c * 128:(c + 1) * 128]

        ec = enc[c * SLOC:(c + 1) * SLOC]
        e4 = ec.reshape(4, 128, 8, 128)
        encT_c = np.ascontiguousarray(e4.transpose(3, 2, 0, 1))
        encN_c = np.ascontiguousarray(e4.transpose(1, 0, 2, 3))

        lo, hi = c * VLOC, (c + 1) * VLOC
        wc = np.zeros((VLOC, 2 * H), dtype=np.float32)
        n_real = min(hi, V) - lo
        if n_real > 0:
            wc[:n_real] = Wo_[lo:lo + n_real]
        main = wc[:12 * NCOL].reshape(12, NCOL, 16, 128).transpose(
            0, 3, 2, 1).astype(np.float16)
        last = wc[12 * NCOL:].reshape(NCOL_LAST, 16, 128).transpose(
            2, 1, 0).astype(np.float16)
        wod_hc = np.ascontiguousarray(main[:, :, 0:8, :])
        wod_cc = np.ascontiguousarray(main[:, :, 8:16, :])
        wod_hl = np.ascontiguousarray(last[:, 0:8, :])
        wod_cl = np.ascontiguousarray(last[:, 8:16, :])
        bo_pad = np.full(VLOC, -60000.0, dtype=np.float32)
        if n_real > 0:
            bo_pad[:n_real] = bo_[lo:lo + n_real]

        in_maps.append({
            "word": word,
            "emb": emb,
            "lchc": lchc_c,
            "rowpack": rowpack,
            "wih": wih_t,
            "whh": whh_t,
            "wa": wa_t,
            "ba_col": ba_col,
            "encT": encT_c,
            "encN": encN_c,
            "wod_h": wod_hc,
            "wod_c": wod_cc,
            "wod_hl": wod_hl,
            "wod_cl": wod_cl,
            "bo_r": np.ascontiguousarray(bo_pad),
        })
    return in_maps


def _run(in_maps, trace=False):
    from concourse.bass_utils import run_bass_kernel_spmd
    global _PROG
    if _PROG is None:
        _PROG = _build_program()
    return run_bass_kernel_spmd(_PROG, in_maps, list(range(C)), trace=trace)


def kernel(word_input, last_context, last_hidden, encoder_outputs,
           embedding, W_ih, W_hh, b_ih, b_hh, Wa, ba, Wo, bo,
           _trace=False, _want_res=False):
    in_maps = _prep_inputs(word_input, last_context, last_hidden,
                           encoder_outputs, embedding, W_ih, W_hh, b_ih, b_hh,
                           Wa, ba, Wo, bo)
    res = _run(in_maps, trace=_trace)
    r0 = res.results[0]

    output = r0["out_logp"][:V].reshape(1, V).astype(np.float32)
    context = r0["out_ctx"].reshape(1, H).astype(np.float32)
    hidden = r0["out_hidden"].reshape(1, 1, H).astype(np.float32)
    aw = r0["out_attw"].reshape(C, 128, 4).transpose(0, 2, 1).reshape(S)
    attention_weights = aw.reshape(1, 1, S).astype(np.float32)
    out = (output, context, hidden, attention_weights)
    if _want_res:
        return out, res
    return out
